# revision 5
# baseline (speedup 1.0000x reference)
"""Trainium2 Bass kernel for GQA sliding-window attention with RoPE.

Model (full problem):
  x [4096, 4096] -> q/k/v projections -> RoPE(q,k) -> GQA sliding-window
  attention (B=2 packed seqs of S=2048, window=1024) -> out proj [4096, 4096].

Sharding over 8 NeuronCores: tensor-parallel over 4 head-groups (8 q-heads /
2 kv-heads per group) x data-parallel over the 2 packed sequences.
core = g*2 + s.  Each core computes a partial out^T [4096, 2048] (its head
group's contribution for its sequence); the host sums the 4 group-partials
per sequence and transposes.

On-core dataflow (feature-major "transposed" activations throughout):
  phase 1: x^T tiles via PE transpose; q^T/k^T = W^T x^T with fused RoPE on
           PSUM eviction; v token-major.
  phase 2: per (q-tile, head): S = q^T.T k^T chunks (only the <=9 key tiles
           inside the causal sliding window), additive mask on the two edge
           tiles, exp on ACT (row-sums for free), scale by 1/l, PE-transpose
           P, PV accumulated over key tiles -> attn^T.
  phase 3: out^T = wo^T attn^T, streamed to DRAM.

All matmuls run as float32r (full-rate fp32 mode, ~1e-4 rounding).
"""

import sys

for _p in ("/opt/trn_rl_repo",):
    if _p not in sys.path:
        sys.path.insert(0, _p)

import numpy as np

import concourse.bass as bass  # noqa: E402
import concourse.mybir as mybir  # noqa: E402
import concourse.tile as tile  # noqa: E402
from concourse import bacc  # noqa: E402
from concourse.bass_utils import run_bass_kernel_spmd  # noqa: E402

F32 = mybir.dt.float32
F32R = mybir.dt.float32r
AF = mybir.ActivationFunctionType
OP = mybir.AluOpType

DIM = 4096
H = 32
KV = 8
HD = 128
B = 2
S = 2048
WINDOW = 1024
NEG = -100.0  # additive mask; exp(-100+s) == 0 to fp32 precision for |s|<~30

G = 4            # tensor-parallel head groups
HQ = H // G      # q heads per core = 8
HKV = KV // G    # kv heads per core = 2
N_CORES = 8

TOK = S          # tokens per core
CHUNK = 512      # phase-1 token chunk
N_CHUNK = TOK // CHUNK
DT = DIM // 128  # 32 dim tiles
QT = TOK // 128  # 16 query tiles
W_KT = WINDOW // 128  # 8

_NC = None


def _build():
    nc = bacc.Bacc(None, target_bir_lowering=False)

    x_d = nc.dram_tensor("x", [TOK, DIM], F32, kind="ExternalInput")
    wq_d = nc.dram_tensor("wq", [HQ, 8, 128, 4, 128], F32R, kind="ExternalInput")
    wk_d = nc.dram_tensor("wk", [HKV, 8, 128, 4, 128], F32R, kind="ExternalInput")
    wv_d = nc.dram_tensor("wv", [8, 128, 4, HKV * 128], F32R, kind="ExternalInput")
    wo_d = nc.dram_tensor("wo", [32, 128, HQ, 128], F32R, kind="ExternalInput")
    cos_d = nc.dram_tensor("cosT", [64, TOK], F32, kind="ExternalInput")
    sin_d = nc.dram_tensor("sinT", [64, TOK], F32, kind="ExternalInput")
    out_d = nc.dram_tensor("outT", [DIM, TOK], F32, kind="ExternalOutput")

    with tile.TileContext(nc) as tc:
        with tc.tile_pool(name="persist", bufs=1) as pp:
            qT = pp.tile([128, HQ, TOK], F32R, tag="qT")
            kT = pp.tile([128, HKV, TOK], F32R, tag="kT")
            vS = pp.tile([128, QT, HKV * 128], F32R, tag="vS")
            csb = pp.tile([128, TOK], F32, tag="csb")  # rows 0:64 cos, 64:128 sin
            ident = pp.tile([128, 128], F32, tag="ident")
            mdiag = pp.tile([128, 128], F32, tag="mdiag")
            mfar = pp.tile([128, 128], F32, tag="mfar")

            nc.sync.dma_start(csb[0:64, :], cos_d[:])
            nc.sync.dma_start(csb[64:128, :], sin_d[:])
            # identity: 1 on diagonal
            nc.gpsimd.memset(ident[:], 0.0)
            nc.gpsimd.affine_select(
                out=ident[:], in_=ident[:], compare_op=OP.not_equal,
                fill=1.0, base=0, pattern=[[-1, 128]], channel_multiplier=1)
            # diag (causal) mask: keep 0 where q>=k, else NEG
            nc.gpsimd.memset(mdiag[:], 0.0)
            nc.gpsimd.affine_select(
                out=mdiag[:], in_=mdiag[:], compare_op=OP.is_ge,
                fill=NEG, base=0, pattern=[[-1, 128]], channel_multiplier=1)
            # far-edge mask: keep 0 where k>q (i.e. -q + k - 1 >= 0), else NEG
            nc.gpsimd.memset(mfar[:], 0.0)
            nc.gpsimd.affine_select(
                out=mfar[:], in_=mfar[:], compare_op=OP.is_ge,
                fill=NEG, base=-1, pattern=[[1, 128]], channel_multiplier=-1)

            # ---------------- phase 1: x^T + QKV (+RoPE) ----------------
            with tc.tile_pool(name="xTp", bufs=1) as xTp, \
                 tc.tile_pool(name="xsb", bufs=2) as xsb_p, \
                 tc.tile_pool(name="wp", bufs=3) as wp, \
                 tc.tile_pool(name="wvp", bufs=2) as wvp, \
                 tc.tile_pool(name="rtmp", bufs=4) as rt_p, \
                 tc.tile_pool(name="ps_qk", bufs=2, space="PSUM") as ps_qk, \
                 tc.tile_pool(name="ps_v", bufs=4, space="PSUM") as ps_v, \
                 tc.tile_pool(name="ps_tp", bufs=2, space="PSUM") as ps_tp:
                for c in range(N_CHUNK):
                    xT = xTp.tile([128, DT, CHUNK], F32R, tag="xT")
                    # build x^T chunk via PE transposes
                    for tt in range(CHUNK // 128):
                        for dh in range(4):
                            xs = xsb_p.tile([128, 1024], F32, tag="xsb")
                            nc.sync.dma_start(
                                xs[:],
                                x_d[(c * 4 + tt) * 128:(c * 4 + tt) * 128 + 128,
                                    dh * 1024:dh * 1024 + 1024])
                            for q4 in range(2):
                                tp = ps_tp.tile([128, 512], F32, tag="tp")
                                for i in range(4):
                                    nc.tensor.transpose(
                                        tp[:, i * 128:i * 128 + 128],
                                        xs[:, (q4 * 4 + i) * 128:(q4 * 4 + i) * 128 + 128],
                                        ident[:])
                                d0 = dh * 8 + q4 * 4
                                nc.any.tensor_copy(
                                    xT[:, d0:d0 + 4, tt * 128:tt * 128 + 128], tp[:])
                    # Q (ft 0..7) and K (ft 8..9), outputs feature-major + RoPE
                    for ft in range(HQ + HKV):
                        ps = ps_qk.tile([128, CHUNK], F32, tag="qk")
                        for dtg in range(8):
                            wt = wp.tile([128, 4, 128], F32R, tag="w")
                            src = wq_d[ft, dtg] if ft < HQ else wk_d[ft - HQ, dtg]
                            nc.sync.dma_start(wt[:], src)
                            for j in range(4):
                                nc.tensor.matmul(
                                    ps[:], wt[:, j, :], xT[:, dtg * 4 + j, :],
                                    start=(dtg == 0 and j == 0),
                                    stop=(dtg == 7 and j == 3))
                        # RoPE eviction (interleaved pairs pre-permuted to
                        # [evens; odds] on host): rows 0:64 = t0, 64:128 = t1
                        if ft < HQ:
                            dst = qT[:, ft, c * CHUNK:(c + 1) * CHUNK]
                        else:
                            dst = kT[:, ft - HQ, c * CHUNK:(c + 1) * CHUNK]
                        cs_ = csb[0:64, c * CHUNK:(c + 1) * CHUNK]
                        sn_ = csb[64:128, c * CHUNK:(c + 1) * CHUNK]
                        t0c = rt_p.tile([64, CHUNK], F32, tag="rt", name=f"t0c_{c}_{ft}")
                        t1s = rt_p.tile([64, CHUNK], F32, tag="rt", name=f"t1s_{c}_{ft}")
                        t0s = rt_p.tile([64, CHUNK], F32, tag="rt", name=f"t0s_{c}_{ft}")
                        t1c = rt_p.tile([64, CHUNK], F32, tag="rt", name=f"t1c_{c}_{ft}")
                        nc.vector.tensor_tensor(t0c[:], ps[0:64, :], cs_, OP.mult)
                        nc.vector.tensor_tensor(t1s[:], ps[64:128, :], sn_, OP.mult)
                        nc.vector.tensor_sub(dst[0:64, :], t0c[:], t1s[:])
                        nc.vector.tensor_tensor(t0s[:], ps[0:64, :], sn_, OP.mult)
                        nc.vector.tensor_tensor(t1c[:], ps[64:128, :], cs_, OP.mult)
                        nc.vector.tensor_add(dst[64:128, :], t1c[:], t0s[:])
                    # V (token-major)
                    psv = [ps_v.tile([128, HKV * 128], F32, tag="psv",
                                     name=f"psv_{c}_{i}") for i in range(4)]
                    for dtg in range(8):
                        wv_t = wvp.tile([128, 4, HKV * 128], F32R, tag="wv")
                        nc.sync.dma_start(wv_t[:], wv_d[dtg])
                        for j in range(4):
                            for t4 in range(4):
                                nc.tensor.matmul(
                                    psv[t4],
                                    xT[:, dtg * 4 + j, t4 * 128:t4 * 128 + 128],
                                    wv_t[:, j, :],
                                    start=(dtg == 0 and j == 0),
                                    stop=(dtg == 7 and j == 3))
                    for t4 in range(4):
                        nc.any.tensor_copy(vS[:, c * 4 + t4, :], psv[t4])

            # ---------------- phase 2: attention ----------------
            with tc.tile_pool(name="attn", bufs=1) as attn_p:
                attnT = attn_p.tile([128, HQ, TOK], F32R, tag="attnT")
                with tc.tile_pool(name="Pp", bufs=3) as Pp, \
                     tc.tile_pool(name="PTp", bufs=1) as PTp, \
                     tc.tile_pool(name="lp", bufs=6) as lp, \
                     tc.tile_pool(name="ps_s", bufs=2, space="PSUM") as ps_s, \
                     tc.tile_pool(name="ps_pt", bufs=1, space="PSUM") as ps_pt, \
                     tc.tile_pool(name="ps_pv", bufs=1, space="PSUM") as ps_pv:
                    for qt in range(QT):
                        kt_lo = max(0, qt - W_KT)
                        n_kt = qt - kt_lo + 1
                        nk = n_kt * 128
                        for kvh in range(HKV):
                            PT = PTp.tile([128, 9, 512], F32R, tag="PT")
                            for hl in range(4):
                                h = kvh * 4 + hl
                                Sp = ps_s.tile([128, 1152], F32, tag="S")
                                a = 0
                                while a < nk:
                                    b = min(a + 512, nk)
                                    nc.tensor.matmul(
                                        Sp[:, a:b],
                                        qT[:, h, qt * 128:qt * 128 + 128],
                                        kT[:, kvh,
                                           kt_lo * 128 + a:kt_lo * 128 + b],
                                        start=True, stop=True)
                                    a = b
                                nc.vector.tensor_add(
                                    Sp[:, nk - 128:nk], Sp[:, nk - 128:nk], mdiag[:])
                                if n_kt == W_KT + 1:
                                    nc.vector.tensor_add(
                                        Sp[:, 0:128], Sp[:, 0:128], mfar[:])
                                P = Pp.tile([128, 1152], F32, tag="P")
                                lsum = lp.tile([128, 1], F32, tag="l")
                                linv = lp.tile([128, 1], F32, tag="linv")
                                nc.scalar.activation(
                                    P[:, :nk], Sp[:, :nk], AF.Exp, accum_out=lsum[:])
                                nc.vector.reciprocal(linv[:], lsum[:])
                                nc.vector.tensor_scalar_mul(P[:, :nk], P[:, :nk], linv[:])
                                for b4 in range(0, n_kt, 4):
                                    cnt = min(4, n_kt - b4)
                                    tpp = ps_pt.tile([128, 512], F32, tag="pt")
                                    for i in range(cnt):
                                        nc.tensor.transpose(
                                            tpp[:, i * 128:i * 128 + 128],
                                            P[:, (b4 + i) * 128:(b4 + i) * 128 + 128],
                                            ident[:])
                                    nc.any.tensor_copy(
                                        PT[:, b4:b4 + cnt, hl * 128:hl * 128 + 128],
                                        tpp[:, :cnt * 128])
                            pv = ps_pv.tile([128, 512], F32, tag="pv")
                            for ki in range(n_kt):
                                nc.tensor.matmul(
                                    pv[:],
                                    vS[:, kt_lo + ki, kvh * 128:kvh * 128 + 128],
                                    PT[:, ki, :],
                                    start=(ki == 0), stop=(ki == n_kt - 1))
                            nc.any.tensor_copy(
                                attnT[:, kvh * 4:kvh * 4 + 4, qt * 128:qt * 128 + 128],
                                pv[:])

                # ---------------- phase 3: output projection ----------------
                with tc.tile_pool(name="wop", bufs=3) as wop, \
                     tc.tile_pool(name="outp", bufs=4) as outp, \
                     tc.tile_pool(name="ps_wo", bufs=2, space="PSUM") as ps_wo:
                    for do in range(32):
                        wt = wop.tile([128, HQ, 128], F32R, tag="wo")
                        nc.sync.dma_start(wt[:], wo_d[do])
                        pso = ps_wo.tile([128, TOK], F32, tag="pso")
                        for ft in range(HQ):
                            for t4 in range(4):
                                nc.tensor.matmul(
                                    pso[:, t4 * 512:t4 * 512 + 512],
                                    wt[:, ft, :],
                                    attnT[:, ft, t4 * 512:t4 * 512 + 512],
                                    start=(ft == 0), stop=(ft == HQ - 1))
                        for t4 in range(4):
                            ob = outp.tile([128, 512], F32, tag="ob")
                            nc.any.tensor_copy(ob[:], pso[:, t4 * 512:t4 * 512 + 512])
                            nc.sync.dma_start(
                                out_d[do * 128:do * 128 + 128,
                                      t4 * 512:t4 * 512 + 512], ob[:])

    nc.compile()
    return nc


def _get_nc():
    global _NC
    if _NC is None:
        _NC = _build()
    return _NC


def _prep_inputs(x, cos, sin, wq, wk, wv, wo):
    """Shard + repack host-side.  Returns in_maps for cores g*2+s."""
    perm = np.concatenate([np.arange(0, HD, 2), np.arange(1, HD, 2)])
    scale = 1.0 / np.sqrt(np.float32(HD))
    # permute interleaved rope pairs to [evens; odds] per head; fold 1/sqrt(hd)
    wq_p = (wq.reshape(DIM, H, HD)[:, :, perm] * scale).astype(np.float32)
    wk_p = wk.reshape(DIM, KV, HD)[:, :, perm].astype(np.float32)
    wv_r = np.ascontiguousarray(wv.reshape(DIM, KV, HD))
    cosT = np.ascontiguousarray(cos[:S].T, dtype=np.float32)
    sinT = np.ascontiguousarray(sin[:S].T, dtype=np.float32)

    in_maps = []
    for g in range(G):
        # [dim, hq, hd] -> [ft, dtg, p, j, c]
        a = wq_p[:, g * HQ:(g + 1) * HQ, :].reshape(8, 4, 128, HQ, 128)
        wq_h = np.ascontiguousarray(a.transpose(3, 0, 2, 1, 4))
        a = wk_p[:, g * HKV:(g + 1) * HKV, :].reshape(8, 4, 128, HKV, 128)
        wk_h = np.ascontiguousarray(a.transpose(3, 0, 2, 1, 4))
        a = wv_r[:, g * HKV:(g + 1) * HKV, :].reshape(8, 4, 128, HKV * 128)
        wv_h = np.ascontiguousarray(a.transpose(0, 2, 1, 3))
        a = wo[g * HQ * HD:(g + 1) * HQ * HD, :].reshape(HQ, 128, 32, 128)
        wo_h = np.ascontiguousarray(a.transpose(2, 1, 0, 3))
        for s in range(B):
            xs = np.ascontiguousarray(x[s * S:(s + 1) * S], dtype=np.float32)
            in_maps.append({
                "x": xs, "wq": wq_h, "wk": wk_h, "wv": wv_h, "wo": wo_h,
                "cosT": cosT, "sinT": sinT,
            })
    return in_maps


def kernel(x, cos, sin, wq, wk, wv, wo, batch=B, window=WINDOW, **_):
    x = np.asarray(x)
    nc = _get_nc()
    in_maps = _prep_inputs(np.asarray(x, np.float32), np.asarray(cos, np.float32),
                           np.asarray(sin, np.float32), np.asarray(wq, np.float32),
                           np.asarray(wk, np.float32), np.asarray(wv, np.float32),
                           np.asarray(wo, np.float32))
    res = run_bass_kernel_spmd(nc, in_maps, core_ids=list(range(N_CORES)))
    out = np.zeros((B * S, DIM), np.float32)
    for g in range(G):
        for s in range(B):
            out[s * S:(s + 1) * S, :] += res.results[g * B + s]["outT"].T
    return out


# revision 10
# speedup vs baseline: 1.0101x; 1.0101x over previous
"""Trainium2 Bass kernel for GQA sliding-window attention with RoPE.

Model (full problem):
  x [4096, 4096] -> q/k/v projections -> RoPE(q,k) -> GQA sliding-window
  attention (B=2 packed seqs of S=2048, window=1024) -> out proj [4096, 4096].

Sharding over 8 NeuronCores: tensor-parallel over 4 head-groups (8 q-heads /
2 kv-heads per group) x data-parallel over the 2 packed sequences.
core = g*2 + s.  Each core computes a partial out^T [4096, 2048] (its head
group's contribution for its sequence); the host sums the 4 group-partials
per sequence and transposes.

On-core dataflow (feature-major "transposed" activations throughout):
  phase 1: x^T tiles via PE transpose; q^T/k^T = W^T x^T with fused RoPE on
           PSUM eviction; v token-major.
  phase 2: per (q-tile, head): S = q^T.T k^T chunks (only the <=9 key tiles
           inside the causal sliding window), additive mask on the two edge
           tiles, exp on ACT (row-sums for free), scale by 1/l, PE-transpose
           P, PV accumulated over key tiles -> attn^T.
  phase 3: out^T = wo^T attn^T, streamed to DRAM.

All matmuls run as float32r (full-rate fp32 mode, ~1e-4 rounding).
"""

import sys

for _p in ("/opt/trn_rl_repo",):
    if _p not in sys.path:
        sys.path.insert(0, _p)

import numpy as np

import concourse.bass as bass  # noqa: E402
import concourse.mybir as mybir  # noqa: E402
import concourse.tile as tile  # noqa: E402
from concourse import bacc  # noqa: E402
from concourse.bass_utils import run_bass_kernel_spmd  # noqa: E402

F32 = mybir.dt.float32
F32R = mybir.dt.float32r
AF = mybir.ActivationFunctionType
OP = mybir.AluOpType

DIM = 4096
H = 32
KV = 8
HD = 128
B = 2
S = 2048
WINDOW = 1024
NEG = -100.0  # additive mask; exp(-100+s) == 0 to fp32 precision for |s|<~30

G = 4            # tensor-parallel head groups
HQ = H // G      # q heads per core = 8
HKV = KV // G    # kv heads per core = 2
N_CORES = 8

TOK = S          # tokens per core
CHUNK = 512      # phase-1 token chunk
N_CHUNK = TOK // CHUNK
DT = DIM // 128  # 32 dim tiles
QT = TOK // 128  # 16 query tiles
W_KT = WINDOW // 128  # 8

_NC = None


def _build():
    nc = bacc.Bacc(None, target_bir_lowering=False)

    x_d = nc.dram_tensor("x", [TOK, DIM], F32R, kind="ExternalInput")
    wq_d = nc.dram_tensor("wq", [HQ, 4, 128, 8, 128], F32R, kind="ExternalInput")
    wk_d = nc.dram_tensor("wk", [HKV, 4, 128, 8, 128], F32R, kind="ExternalInput")
    wv_d = nc.dram_tensor("wv", [8, 128, 4, HKV * 128], F32R, kind="ExternalInput")
    wo_d = nc.dram_tensor("wo", [32, 128, HQ, 128], F32R, kind="ExternalInput")
    cos_d = nc.dram_tensor("cosT", [64, TOK], F32, kind="ExternalInput")
    sin_d = nc.dram_tensor("sinT", [64, TOK], F32, kind="ExternalInput")
    out_d = nc.dram_tensor("outT", [DIM, TOK], F32, kind="ExternalOutput")

    with tile.TileContext(nc) as tc:
        with tc.tile_pool(name="persist", bufs=1) as pp:
            qT = pp.tile([128, HQ, TOK], F32R, tag="qT")
            kT = pp.tile([128, HKV, TOK], F32R, tag="kT")
            vS = pp.tile([128, QT, HKV * 128], F32R, tag="vS")
            ident = pp.tile([128, 128], F32, tag="ident")
            ident_r = pp.tile([128, 128], F32R, tag="identr")
            mdiagT = pp.tile([128, 128], F32, tag="mdiagT")
            mfarT = pp.tile([128, 128], F32, tag="mfarT")
            ones_r = pp.tile([128, 1], F32R, tag="ones_r")
            zeros_r = pp.tile([128, 128], F32R, tag="zeros_r")

            # identity: 1 on diagonal
            nc.gpsimd.memset(ident[:], 0.0)
            nc.gpsimd.affine_select(
                out=ident[:], in_=ident[:], compare_op=OP.not_equal,
                fill=1.0, base=0, pattern=[[-1, 128]], channel_multiplier=1)
            nc.vector.tensor_copy(ident_r[:], ident[:])
            # S^T orientation [k(part), q(free)] masks:
            # diag block: keep 0 where q >= k  (-k + q >= 0)
            nc.gpsimd.memset(mdiagT[:], 0.0)
            nc.gpsimd.affine_select(
                out=mdiagT[:], in_=mdiagT[:], compare_op=OP.is_ge,
                fill=NEG, base=0, pattern=[[1, 128]], channel_multiplier=-1)
            # far-edge block: keep 0 where q < k  (k - q - 1 >= 0)
            nc.gpsimd.memset(mfarT[:], 0.0)
            nc.gpsimd.affine_select(
                out=mfarT[:], in_=mfarT[:], compare_op=OP.is_ge,
                fill=NEG, base=-1, pattern=[[-1, 128]], channel_multiplier=1)
            ones_f = pp.tile([128, 1], F32, tag="ones_f")
            zeros_f = pp.tile([128, 128], F32, tag="zeros_f")
            nc.vector.memset(ones_f[:], 1.0)
            nc.vector.memset(zeros_f[:], 0.0)
            nc.vector.tensor_copy(ones_r[:], ones_f[:])
            nc.vector.tensor_copy(zeros_r[:], zeros_f[:])

            # ---------------- phase 1: x^T + QKV (+RoPE) ----------------
            with tc.tile_pool(name="xTp", bufs=1) as xTp, \
                 tc.tile_pool(name="xsb", bufs=2) as xsb_p, \
                 tc.tile_pool(name="wp", bufs=3) as wp, \
                 tc.tile_pool(name="wvp", bufs=2) as wvp, \
                 tc.tile_pool(name="rtmp", bufs=4) as rt_p, \
                 tc.tile_pool(name="ps_qk", bufs=2, space="PSUM") as ps_qk, \
                 tc.tile_pool(name="ps_v", bufs=4, space="PSUM") as ps_v, \
                 tc.tile_pool(name="ps_tp", bufs=2, space="PSUM") as ps_tp:
                csb = xTp.tile([128, TOK], F32, tag="csb")  # rows 0:64 cos, 64:128 sin
                nc.sync.dma_start(csb[0:64, :], cos_d[:])
                nc.sync.dma_start(csb[64:128, :], sin_d[:])
                for c in range(N_CHUNK):
                    xT = xTp.tile([128, DT, CHUNK], F32R, tag="xT")
                    # build x^T chunk via PE transposes
                    for tt in range(CHUNK // 128):
                        for dh in range(4):
                            xs = xsb_p.tile([128, 1024], F32R, tag="xsb")
                            nc.sync.dma_start(
                                xs[:],
                                x_d[(c * 4 + tt) * 128:(c * 4 + tt) * 128 + 128,
                                    dh * 1024:dh * 1024 + 1024])
                            for q4 in range(2):
                                tp = ps_tp.tile([128, 512], F32R, tag="tp")
                                for i in range(4):
                                    nc.tensor.transpose(
                                        tp[:, i * 128:i * 128 + 128],
                                        xs[:, (q4 * 4 + i) * 128:(q4 * 4 + i) * 128 + 128],
                                        ident_r[:])
                                d0 = dh * 8 + q4 * 4
                                nc.any.tensor_copy(
                                    xT[:, d0:d0 + 4, tt * 128:tt * 128 + 128], tp[:])
                    # Q (ft 0..7) and K (ft 8..9), outputs feature-major + RoPE
                    for ft in range(HQ + HKV):
                        ps = ps_qk.tile([128, CHUNK], F32, tag="qk")
                        for dtg in range(4):
                            wt = wp.tile([128, 8, 128], F32R, tag="w")
                            src = wq_d[ft, dtg] if ft < HQ else wk_d[ft - HQ, dtg]
                            nc.sync.dma_start(wt[:], src)
                            for j in range(8):
                                nc.tensor.matmul(
                                    ps[:], wt[:, j, :], xT[:, dtg * 8 + j, :],
                                    start=(dtg == 0 and j == 0),
                                    stop=(dtg == 3 and j == 7))
                        # RoPE eviction (interleaved pairs pre-permuted to
                        # [evens; odds] on host): rows 0:64 = t0, 64:128 = t1
                        if ft < HQ:
                            dst = qT[:, ft, c * CHUNK:(c + 1) * CHUNK]
                        else:
                            dst = kT[:, ft - HQ, c * CHUNK:(c + 1) * CHUNK]
                        cs_ = csb[0:64, c * CHUNK:(c + 1) * CHUNK]
                        sn_ = csb[64:128, c * CHUNK:(c + 1) * CHUNK]
                        t0c = rt_p.tile([64, CHUNK], F32, tag="rt", name=f"t0c_{c}_{ft}")
                        t1s = rt_p.tile([64, CHUNK], F32, tag="rt", name=f"t1s_{c}_{ft}")
                        t0s = rt_p.tile([64, CHUNK], F32, tag="rt", name=f"t0s_{c}_{ft}")
                        t1c = rt_p.tile([64, CHUNK], F32, tag="rt", name=f"t1c_{c}_{ft}")
                        nc.vector.tensor_tensor(t0c[:], ps[0:64, :], cs_, OP.mult)
                        nc.vector.tensor_tensor(t1s[:], ps[64:128, :], sn_, OP.mult)
                        nc.vector.tensor_sub(dst[0:64, :], t0c[:], t1s[:])
                        nc.vector.tensor_tensor(t0s[:], ps[0:64, :], sn_, OP.mult)
                        nc.vector.tensor_tensor(t1c[:], ps[64:128, :], cs_, OP.mult)
                        nc.vector.tensor_add(dst[64:128, :], t1c[:], t0s[:])
                    # V (token-major)
                    psv = [ps_v.tile([128, HKV * 128], F32, tag="psv",
                                     name=f"psv_{c}_{i}") for i in range(4)]
                    for dtg in range(8):
                        wv_t = wvp.tile([128, 4, HKV * 128], F32R, tag="wv")
                        nc.sync.dma_start(wv_t[:], wv_d[dtg])
                        for j in range(4):
                            for t4 in range(4):
                                nc.tensor.matmul(
                                    psv[t4],
                                    xT[:, dtg * 4 + j, t4 * 128:t4 * 128 + 128],
                                    wv_t[:, j, :],
                                    start=(dtg == 0 and j == 0),
                                    stop=(dtg == 7 and j == 3))
                    for t4 in range(4):
                        nc.any.tensor_copy(vS[:, c * 4 + t4, :], psv[t4])

            # ---------------- phase 2: attention (S^T orientation) ----------
            # Per (head h, key-tile kt): S^T[k, q] for the q-window
            # [kt*128, (kt+9)*128) that kt participates in.  exp on ACT gives
            # P^T directly (no transposes).  PV accumulates over kt into
            # out^T psum per 512-token column block qc; row-sums l accumulate
            # in psum via ones-matmuls.  Eviction divides by l (broadcast via
            # GPSIMD) and writes attnT.
            with tc.tile_pool(name="attn", bufs=1) as attn_p:
                attnT = attn_p.tile([128, HQ, TOK], F32R, tag="attnT")
                with tc.tile_pool(name="PTk", bufs=3) as PTkp, \
                     tc.tile_pool(name="lts", bufs=4) as ltsp, \
                     tc.tile_pool(name="lbp", bufs=4) as lbp, \
                     tc.tile_pool(name="ps_s", bufs=2, space="PSUM") as ps_s, \
                     tc.tile_pool(name="ps_o", bufs=3, space="PSUM") as ps_o, \
                     tc.tile_pool(name="ps_l", bufs=3, space="PSUM") as ps_l:
                    NQC = TOK // 512  # 4 column blocks
                    for h in range(HQ):
                        kvh = h // 4
                        outp = {}
                        lps = {}
                        for kt in range(QT):
                            qlo, qhi = kt * 128, min((kt + 9) * 128, TOK)
                            # lazily zero-init accumulators for newly covered qc
                            for qc in range((qlo // 512), (qhi + 511) // 512):
                                if qc not in outp:
                                    o = ps_o.tile([128, 512], F32, tag="outp",
                                                  name=f"outp_{h}_{qc}")
                                    lq = ps_l.tile([1, 512], F32, tag="lps",
                                                   name=f"lps_{h}_{qc}")
                                    nc.tensor.matmul(
                                        o[:], zeros_r[:], qT[:, 0, 0:512],
                                        start=True, stop=False,
                                        skip_group_check=True)
                                    nc.tensor.matmul(
                                        lq[:], zeros_r[:, 0:1], qT[:, 0, 0:512],
                                        start=True, stop=False,
                                        skip_group_check=True)
                                    outp[qc] = o
                                    lps[qc] = lq
                            # S^T pieces, split at absolute 512 boundaries
                            bounds = [qlo]
                            nb = (qlo // 512 + 1) * 512
                            while nb < qhi:
                                bounds.append(nb)
                                nb += 512
                            bounds.append(qhi)
                            PTk = PTkp.tile([128, 1152], F32R, tag="PTk",
                                            name=f"PTk_{h}_{kt}")
                            for a, b in zip(bounds[:-1], bounds[1:]):
                                Sp = ps_s.tile([128, 512], F32, tag="S",
                                               name=f"S_{h}_{kt}_{a}")
                                nc.tensor.matmul(
                                    Sp[:, :b - a],
                                    kT[:, kvh, kt * 128:kt * 128 + 128],
                                    qT[:, h, a:b],
                                    start=True, stop=True)
                                if a == qlo:  # causal diagonal block
                                    nc.vector.tensor_add(
                                        Sp[:, 0:128], Sp[:, 0:128], mdiagT[:])
                                if b == qhi and kt + 8 < QT:  # window far edge
                                    nc.vector.tensor_add(
                                        Sp[:, qhi - 128 - a:qhi - a],
                                        Sp[:, qhi - 128 - a:qhi - a], mfarT[:])
                                nc.scalar.activation(
                                    PTk[:, a - qlo:b - qlo], Sp[:, :b - a], AF.Exp)
                            # PV + l accumulation per piece
                            for a, b in zip(bounds[:-1], bounds[1:]):
                                qc = a // 512
                                last = (kt == min(QT - 1, 4 * qc + 3))
                                nc.tensor.matmul(
                                    outp[qc][:, a - qc * 512:b - qc * 512],
                                    vS[:, kt, kvh * 128:kvh * 128 + 128],
                                    PTk[:, a - qlo:b - qlo],
                                    start=False, stop=last,
                                    skip_group_check=True)
                                nc.tensor.matmul(
                                    lps[qc][:, a - qc * 512:b - qc * 512],
                                    ones_r[:],
                                    PTk[:, a - qlo:b - qlo],
                                    start=False, stop=last,
                                    skip_group_check=True)
                            # evict any qc whose last writer was this kt
                            for qc in list(outp.keys()):
                                if kt == min(QT - 1, 4 * qc + 3):
                                    lts = ltsp.tile([1, 512], F32, tag="lts",
                                                    name=f"lts_{h}_{qc}")
                                    nc.vector.tensor_copy(lts[:], lps[qc][:])
                                    nc.vector.reciprocal(lts[:], lts[:])
                                    lb = lbp.tile([128, 512], F32, tag="lb",
                                                  name=f"lb_{h}_{qc}")
                                    nc.gpsimd.partition_broadcast(lb[:], lts[:])
                                    nc.vector.tensor_tensor(
                                        attnT[:, h, qc * 512:qc * 512 + 512],
                                        outp[qc][:], lb[:], OP.mult)
                                    del outp[qc]
                                    del lps[qc]

                # ---------------- phase 3: output projection ----------------
                with tc.tile_pool(name="wop", bufs=3) as wop, \
                     tc.tile_pool(name="outp", bufs=4) as outp, \
                     tc.tile_pool(name="ps_wo", bufs=2, space="PSUM") as ps_wo:
                    for do in range(32):
                        wt = wop.tile([128, HQ, 128], F32R, tag="wo")
                        nc.sync.dma_start(wt[:], wo_d[do])
                        pso = ps_wo.tile([128, TOK], F32, tag="pso")
                        for ft in range(HQ):
                            for t4 in range(4):
                                nc.tensor.matmul(
                                    pso[:, t4 * 512:t4 * 512 + 512],
                                    wt[:, ft, :],
                                    attnT[:, ft, t4 * 512:t4 * 512 + 512],
                                    start=(ft == 0), stop=(ft == HQ - 1))
                        for t4 in range(4):
                            ob = outp.tile([128, 512], F32, tag="ob")
                            nc.any.tensor_copy(ob[:], pso[:, t4 * 512:t4 * 512 + 512])
                            nc.sync.dma_start(
                                out_d[do * 128:do * 128 + 128,
                                      t4 * 512:t4 * 512 + 512], ob[:])

    nc.compile()
    return nc


def _get_nc():
    global _NC
    if _NC is None:
        _NC = _build()
    return _NC


def _prep_inputs(x, cos, sin, wq, wk, wv, wo):
    """Shard + repack host-side.  Returns in_maps for cores g*2+s."""
    perm = np.concatenate([np.arange(0, HD, 2), np.arange(1, HD, 2)])
    scale = 1.0 / np.sqrt(np.float32(HD))
    # permute interleaved rope pairs to [evens; odds] per head; fold 1/sqrt(hd)
    wq_p = (wq.reshape(DIM, H, HD)[:, :, perm] * scale).astype(np.float32)
    wk_p = wk.reshape(DIM, KV, HD)[:, :, perm].astype(np.float32)
    wv_r = np.ascontiguousarray(wv.reshape(DIM, KV, HD))
    cosT = np.ascontiguousarray(cos[:S].T, dtype=np.float32)
    sinT = np.ascontiguousarray(sin[:S].T, dtype=np.float32)

    in_maps = []
    for g in range(G):
        # [dim, hq, hd] -> [ft, dtg, p, j, c]
        a = wq_p[:, g * HQ:(g + 1) * HQ, :].reshape(4, 8, 128, HQ, 128)
        wq_h = np.ascontiguousarray(a.transpose(3, 0, 2, 1, 4))
        a = wk_p[:, g * HKV:(g + 1) * HKV, :].reshape(4, 8, 128, HKV, 128)
        wk_h = np.ascontiguousarray(a.transpose(3, 0, 2, 1, 4))
        a = wv_r[:, g * HKV:(g + 1) * HKV, :].reshape(8, 4, 128, HKV * 128)
        wv_h = np.ascontiguousarray(a.transpose(0, 2, 1, 3))
        a = wo[g * HQ * HD:(g + 1) * HQ * HD, :].reshape(HQ, 128, 32, 128)
        wo_h = np.ascontiguousarray(a.transpose(2, 1, 0, 3))
        for s in range(B):
            xs = np.ascontiguousarray(x[s * S:(s + 1) * S], dtype=np.float32)
            in_maps.append({
                "x": xs, "wq": wq_h, "wk": wk_h, "wv": wv_h, "wo": wo_h,
                "cosT": cosT, "sinT": sinT,
            })
    return in_maps


def kernel(x, cos, sin, wq, wk, wv, wo, batch=B, window=WINDOW, **_):
    x = np.asarray(x)
    nc = _get_nc()
    in_maps = _prep_inputs(np.asarray(x, np.float32), np.asarray(cos, np.float32),
                           np.asarray(sin, np.float32), np.asarray(wq, np.float32),
                           np.asarray(wk, np.float32), np.asarray(wv, np.float32),
                           np.asarray(wo, np.float32))
    res = run_bass_kernel_spmd(nc, in_maps, core_ids=list(range(N_CORES)))
    out = np.zeros((B * S, DIM), np.float32)
    for g in range(G):
        for s in range(B):
            out[s * S:(s + 1) * S, :] += res.results[g * B + s]["outT"].T
    return out


# revision 15
# speedup vs baseline: 1.0652x; 1.0546x over previous
"""Trainium2 Bass kernel for GQA sliding-window attention with RoPE.

Model (full problem):
  x [4096, 4096] -> q/k/v projections -> RoPE(q,k) -> GQA sliding-window
  attention (B=2 packed seqs of S=2048, window=1024) -> out proj [4096, 4096].

Sharding over 8 NeuronCores: tensor-parallel over 4 head-groups (8 q-heads /
2 kv-heads per group) x data-parallel over the 2 packed sequences.
core = g*2 + s.  Each core computes a partial out^T [4096, 2048] (its head
group's contribution for its sequence); the host sums the 4 group-partials
per sequence and transposes.

On-core dataflow (feature-major "transposed" activations throughout):
  phase 1: x^T tiles via PE transpose; q^T/k^T = W^T x^T with fused RoPE on
           PSUM eviction; v token-major.
  phase 2: per (q-tile, head): S = q^T.T k^T chunks (only the <=9 key tiles
           inside the causal sliding window), additive mask on the two edge
           tiles, exp on ACT (row-sums for free), scale by 1/l, PE-transpose
           P, PV accumulated over key tiles -> attn^T.
  phase 3: out^T = wo^T attn^T, streamed to DRAM.

All matmuls run as float32r (full-rate fp32 mode, ~1e-4 rounding).
"""

import sys

for _p in ("/opt/trn_rl_repo",):
    if _p not in sys.path:
        sys.path.insert(0, _p)

import numpy as np

import concourse.bass as bass  # noqa: E402
import concourse.mybir as mybir  # noqa: E402
import concourse.tile as tile  # noqa: E402
from concourse import bacc  # noqa: E402
from concourse.bass_utils import run_bass_kernel_spmd  # noqa: E402

F32 = mybir.dt.float32
F32R = mybir.dt.float32r
AF = mybir.ActivationFunctionType
OP = mybir.AluOpType

DIM = 4096
H = 32
KV = 8
HD = 128
B = 2
S = 2048
WINDOW = 1024
NEG = -100.0  # additive mask; exp(-100+s) == 0 to fp32 precision for |s|<~30

G = 4            # tensor-parallel head groups
HQ = H // G      # q heads per core = 8
HKV = KV // G    # kv heads per core = 2
N_CORES = 8

TOK = S          # tokens per core
CHUNK = 512      # phase-1 token chunk
N_CHUNK = TOK // CHUNK
DT = DIM // 128  # 32 dim tiles
QT = TOK // 128  # 16 query tiles
W_KT = WINDOW // 128  # 8

_NC = None


def _build():
    nc = bacc.Bacc(None, target_bir_lowering=False)

    xT_d = nc.dram_tensor("xT", [DIM, TOK], F32R, kind="ExternalInput")
    wq_d = nc.dram_tensor("wq", [HQ, 4, 128, 8, 128], F32R, kind="ExternalInput")
    wk_d = nc.dram_tensor("wk", [HKV, 4, 128, 8, 128], F32R, kind="ExternalInput")
    wv_d = nc.dram_tensor("wv", [8, 128, 4, HKV * 128], F32R, kind="ExternalInput")
    wo_d = nc.dram_tensor("wo", [32, 128, HQ, 128], F32R, kind="ExternalInput")
    cos_d = nc.dram_tensor("cosT", [64, TOK], F32, kind="ExternalInput")
    sin_d = nc.dram_tensor("sinT", [64, TOK], F32, kind="ExternalInput")
    out_d = nc.dram_tensor("outT", [DIM, TOK], F32, kind="ExternalOutput")

    with tile.TileContext(nc) as tc:
        with tc.tile_pool(name="persist", bufs=1) as pp:
            qT = pp.tile([128, HQ, TOK], F32R, tag="qT")
            kT = pp.tile([128, HKV, TOK], F32R, tag="kT")
            vS = pp.tile([128, QT, HKV * 128], F32R, tag="vS")
            mdiagT = pp.tile([128, 128], F32, tag="mdiagT")
            mfarT = pp.tile([128, 128], F32, tag="mfarT")
            ones_r = pp.tile([128, 1], F32R, tag="ones_r")
            zeros_r = pp.tile([128, 128], F32R, tag="zeros_r")

            # S^T orientation [k(part), q(free)] masks:
            # diag block: keep 0 where q >= k  (-k + q >= 0)
            nc.gpsimd.memset(mdiagT[:], 0.0)
            nc.gpsimd.affine_select(
                out=mdiagT[:], in_=mdiagT[:], compare_op=OP.is_ge,
                fill=NEG, base=0, pattern=[[1, 128]], channel_multiplier=-1)
            # far-edge block: keep 0 where q < k  (k - q - 1 >= 0)
            nc.gpsimd.memset(mfarT[:], 0.0)
            nc.gpsimd.affine_select(
                out=mfarT[:], in_=mfarT[:], compare_op=OP.is_ge,
                fill=NEG, base=-1, pattern=[[-1, 128]], channel_multiplier=1)
            ones_f = pp.tile([128, 1], F32, tag="ones_f")
            zeros_f = pp.tile([128, 128], F32, tag="zeros_f")
            nc.vector.memset(ones_f[:], 1.0)
            nc.vector.memset(zeros_f[:], 0.0)
            nc.vector.tensor_copy(ones_r[:], ones_f[:])
            nc.vector.tensor_copy(zeros_r[:], zeros_f[:])

            # ---------------- phase 1: QKV (+RoPE) -------------------------
            # x^T comes pre-transposed from the host.  Per 512-token chunk,
            # x^T tiles stream into a per-dim-tile ring; q/k features run in
            # groups of <=3 with dim-tile-outer loops so the next chunk's
            # x^T DMAs overlap the tail groups.  wk/wv stay resident.
            with tc.tile_pool(name="xTr", bufs=32) as xTr, \
                 tc.tile_pool(name="wvs", bufs=2) as wvs, \
                 tc.tile_pool(name="wqs", bufs=5) as wqs, \
                 tc.tile_pool(name="csp", bufs=1) as csp, \
                 tc.tile_pool(name="rtmp", bufs=3) as rt_p, \
                 tc.tile_pool(name="ps_qk", bufs=4, space="PSUM") as ps_qk, \
                 tc.tile_pool(name="ps_v", bufs=4, space="PSUM") as ps_v:
                csb = csp.tile([128, TOK], F32, tag="csb")  # 0:64 cos, 64:128 sin
                nc.gpsimd.dma_start(csb[0:64, :], cos_d[:])
                nc.gpsimd.dma_start(csb[64:128, :], sin_d[:])

                GROUPS = [(0, 1, 2), (3, 4, 5), (6, 7, 8), (9,)]  # ft 8/9 = k0/k1

                def rope_evict(ps, ft, c):
                    if ft < HQ:
                        dst = qT[:, ft, c * CHUNK:(c + 1) * CHUNK]
                    else:
                        dst = kT[:, ft - HQ, c * CHUNK:(c + 1) * CHUNK]
                    cs_ = csb[0:64, c * CHUNK:(c + 1) * CHUNK]
                    sn_ = csb[64:128, c * CHUNK:(c + 1) * CHUNK]
                    t0c = rt_p.tile([64, CHUNK], F32, tag="rt", name=f"t0c_{c}_{ft}")
                    t1s = rt_p.tile([64, CHUNK], F32, tag="rt", name=f"t1s_{c}_{ft}")
                    t0s = rt_p.tile([64, CHUNK], F32, tag="rt", name=f"t0s_{c}_{ft}")
                    t1c = rt_p.tile([64, CHUNK], F32, tag="rt", name=f"t1c_{c}_{ft}")
                    nc.any.tensor_tensor(t0c[:], ps[0:64, :], cs_, OP.mult)
                    nc.any.tensor_tensor(t1s[:], ps[64:128, :], sn_, OP.mult)
                    nc.any.tensor_sub(dst[0:64, :], t0c[:], t1s[:])
                    nc.any.tensor_tensor(t0s[:], ps[0:64, :], sn_, OP.mult)
                    nc.any.tensor_tensor(t1c[:], ps[64:128, :], cs_, OP.mult)
                    nc.any.tensor_add(dst[64:128, :], t1c[:], t0s[:])

                for c in range(N_CHUNK):
                    xTt = []
                    for dt in range(DT):
                        t = xTr.tile([128, CHUNK], F32R, tag="xT",
                                     name=f"xT_{c}_{dt}")
                        nc.gpsimd.dma_start(
                            t[:], xT_d[dt * 128:dt * 128 + 128,
                                       c * CHUNK:(c + 1) * CHUNK])
                        xTt.append(t)
                    for grp in GROUPS:
                        pss = {ft: ps_qk.tile([128, CHUNK], F32, tag="qk",
                                              name=f"qk_{c}_{ft}")
                               for ft in grp}
                        for dtg in range(4):
                            wts = {}
                            for ft in grp:
                                wt = wqs.tile([128, 8, 128], F32R, tag="w",
                                              name=f"w_{c}_{ft}_{dtg}")
                                src_ = (wq_d[ft, dtg] if ft < HQ
                                        else wk_d[ft - HQ, dtg])
                                nc.sync.dma_start(wt[:], src_)
                                wts[ft] = wt
                            for j in range(8):
                                dt = dtg * 8 + j
                                for ft in grp:
                                    nc.tensor.matmul(
                                        pss[ft][:], wts[ft][:, j, :], xTt[dt][:],
                                        start=(dtg == 0 and j == 0),
                                        stop=(dtg == 3 and j == 7))
                        for ft in grp:
                            rope_evict(pss[ft], ft, c)
                    # V (token-major)
                    psv = [ps_v.tile([128, HKV * 128], F32, tag="psv",
                                     name=f"psv_{c}_{i}") for i in range(4)]
                    for dtg in range(8):
                        wv_t = wvs.tile([128, 4, HKV * 128], F32R, tag="wv",
                                        name=f"wv_{c}_{dtg}")
                        nc.scalar.dma_start(wv_t[:], wv_d[dtg])
                        for j in range(4):
                            dt = dtg * 4 + j
                            for t4 in range(4):
                                nc.tensor.matmul(
                                    psv[t4],
                                    xTt[dt][:, t4 * 128:t4 * 128 + 128],
                                    wv_t[:, j, :],
                                    start=(dt == 0), stop=(dt == DT - 1))
                    for t4 in range(4):
                        nc.any.tensor_copy(vS[:, c * 4 + t4, :], psv[t4])

            # ---------------- phase 2: attention (S^T orientation) ----------
            # Per (head h, key-tile kt): S^T[k, q] for the q-window
            # [kt*128, (kt+9)*128) that kt participates in.  exp on ACT gives
            # P^T directly (no transposes).  PV accumulates over kt into
            # out^T psum per 512-token column block qc; row-sums l accumulate
            # in psum via ones-matmuls.  Eviction divides by l (broadcast via
            # GPSIMD) and writes attnT.
            with tc.tile_pool(name="attn", bufs=1) as attn_p:
                attnT = attn_p.tile([128, HQ, TOK], F32R, tag="attnT")
                with tc.tile_pool(name="PTk", bufs=3) as PTkp, \
                     tc.tile_pool(name="lts", bufs=4) as ltsp, \
                     tc.tile_pool(name="lbp", bufs=4) as lbp, \
                     tc.tile_pool(name="ps_s", bufs=2, space="PSUM") as ps_s, \
                     tc.tile_pool(name="ps_o", bufs=3, space="PSUM") as ps_o, \
                     tc.tile_pool(name="ps_l", bufs=3, space="PSUM") as ps_l:
                    NQC = TOK // 512  # 4 column blocks
                    for h in range(HQ):
                        kvh = h // 4
                        outp = {}
                        lps = {}
                        for kt in range(QT):
                            qlo, qhi = kt * 128, min((kt + 9) * 128, TOK)
                            # lazily zero-init accumulators for newly covered qc
                            for qc in range((qlo // 512), (qhi + 511) // 512):
                                if qc not in outp:
                                    o = ps_o.tile([128, 512], F32, tag="outp",
                                                  name=f"outp_{h}_{qc}")
                                    lq = ps_l.tile([1, 512], F32, tag="lps",
                                                   name=f"lps_{h}_{qc}")
                                    nc.tensor.matmul(
                                        o[:], zeros_r[:], qT[:, 0, 0:512],
                                        start=True, stop=False,
                                        skip_group_check=True)
                                    nc.tensor.matmul(
                                        lq[:], zeros_r[:, 0:1], qT[:, 0, 0:512],
                                        start=True, stop=False,
                                        skip_group_check=True)
                                    outp[qc] = o
                                    lps[qc] = lq
                            # S^T pieces, split at absolute 512 boundaries
                            bounds = [qlo]
                            nb = (qlo // 512 + 1) * 512
                            while nb < qhi:
                                bounds.append(nb)
                                nb += 512
                            bounds.append(qhi)
                            PTk = PTkp.tile([128, 1152], F32R, tag="PTk",
                                            name=f"PTk_{h}_{kt}")
                            for a, b in zip(bounds[:-1], bounds[1:]):
                                Sp = ps_s.tile([128, 512], F32, tag="S",
                                               name=f"S_{h}_{kt}_{a}")
                                nc.tensor.matmul(
                                    Sp[:, :b - a],
                                    kT[:, kvh, kt * 128:kt * 128 + 128],
                                    qT[:, h, a:b],
                                    start=True, stop=True)
                                if a == qlo:  # causal diagonal block
                                    nc.vector.tensor_add(
                                        Sp[:, 0:128], Sp[:, 0:128], mdiagT[:])
                                if b == qhi and kt + 8 < QT:  # window far edge
                                    nc.vector.tensor_add(
                                        Sp[:, qhi - 128 - a:qhi - a],
                                        Sp[:, qhi - 128 - a:qhi - a], mfarT[:])
                                nc.scalar.activation(
                                    PTk[:, a - qlo:b - qlo], Sp[:, :b - a], AF.Exp)
                            # PV + l accumulation per piece
                            for a, b in zip(bounds[:-1], bounds[1:]):
                                qc = a // 512
                                last = (kt == min(QT - 1, 4 * qc + 3))
                                nc.tensor.matmul(
                                    outp[qc][:, a - qc * 512:b - qc * 512],
                                    vS[:, kt, kvh * 128:kvh * 128 + 128],
                                    PTk[:, a - qlo:b - qlo],
                                    start=False, stop=last,
                                    skip_group_check=True)
                                nc.tensor.matmul(
                                    lps[qc][:, a - qc * 512:b - qc * 512],
                                    ones_r[:],
                                    PTk[:, a - qlo:b - qlo],
                                    start=False, stop=last,
                                    skip_group_check=True)
                            # evict any qc whose last writer was this kt
                            for qc in list(outp.keys()):
                                if kt == min(QT - 1, 4 * qc + 3):
                                    lts = ltsp.tile([1, 512], F32, tag="lts",
                                                    name=f"lts_{h}_{qc}")
                                    nc.vector.tensor_copy(lts[:], lps[qc][:])
                                    nc.vector.reciprocal(lts[:], lts[:])
                                    lb = lbp.tile([128, 512], F32, tag="lb",
                                                  name=f"lb_{h}_{qc}")
                                    nc.gpsimd.partition_broadcast(lb[:], lts[:])
                                    nc.vector.tensor_tensor(
                                        attnT[:, h, qc * 512:qc * 512 + 512],
                                        outp[qc][:], lb[:], OP.mult)
                                    del outp[qc]
                                    del lps[qc]

                # ---------------- phase 3: output projection ----------------
                with tc.tile_pool(name="wop", bufs=3) as wop, \
                     tc.tile_pool(name="outp", bufs=4) as outp, \
                     tc.tile_pool(name="ps_wo", bufs=2, space="PSUM") as ps_wo:
                    for do in range(32):
                        wt = wop.tile([128, HQ, 128], F32R, tag="wo")
                        nc.sync.dma_start(wt[:], wo_d[do])
                        pso = ps_wo.tile([128, TOK], F32, tag="pso")
                        for ft in range(HQ):
                            for t4 in range(4):
                                nc.tensor.matmul(
                                    pso[:, t4 * 512:t4 * 512 + 512],
                                    wt[:, ft, :],
                                    attnT[:, ft, t4 * 512:t4 * 512 + 512],
                                    start=(ft == 0), stop=(ft == HQ - 1))
                        for t4 in range(4):
                            ob = outp.tile([128, 512], F32, tag="ob")
                            nc.vector.tensor_copy(ob[:], pso[:, t4 * 512:t4 * 512 + 512])
                            nc.scalar.dma_start(
                                out_d[do * 128:do * 128 + 128,
                                      t4 * 512:t4 * 512 + 512], ob[:])

    nc.compile()
    return nc


def _get_nc():
    global _NC
    if _NC is None:
        _NC = _build()
    return _NC


def _prep_inputs(x, cos, sin, wq, wk, wv, wo):
    """Shard + repack host-side.  Returns in_maps for cores g*2+s."""
    perm = np.concatenate([np.arange(0, HD, 2), np.arange(1, HD, 2)])
    scale = 1.0 / np.sqrt(np.float32(HD))
    # permute interleaved rope pairs to [evens; odds] per head; fold 1/sqrt(hd)
    wq_p = (wq.reshape(DIM, H, HD)[:, :, perm] * scale).astype(np.float32)
    wk_p = wk.reshape(DIM, KV, HD)[:, :, perm].astype(np.float32)
    wv_r = np.ascontiguousarray(wv.reshape(DIM, KV, HD))
    cosT = np.ascontiguousarray(cos[:S].T, dtype=np.float32)
    sinT = np.ascontiguousarray(sin[:S].T, dtype=np.float32)

    in_maps = []
    for g in range(G):
        # [dim, hq, hd] -> [ft, dtg, p, j, c]
        a = wq_p[:, g * HQ:(g + 1) * HQ, :].reshape(4, 8, 128, HQ, 128)
        wq_h = np.ascontiguousarray(a.transpose(3, 0, 2, 1, 4))
        a = wk_p[:, g * HKV:(g + 1) * HKV, :].reshape(4, 8, 128, HKV, 128)
        wk_h = np.ascontiguousarray(a.transpose(3, 0, 2, 1, 4))
        a = wv_r[:, g * HKV:(g + 1) * HKV, :].reshape(8, 4, 128, HKV * 128)
        wv_h = np.ascontiguousarray(a.transpose(0, 2, 1, 3))
        a = wo[g * HQ * HD:(g + 1) * HQ * HD, :].reshape(HQ, 128, 32, 128)
        wo_h = np.ascontiguousarray(a.transpose(2, 1, 0, 3))
        for s in range(B):
            xs = np.ascontiguousarray(x[s * S:(s + 1) * S].T, dtype=np.float32)
            in_maps.append({
                "xT": xs, "wq": wq_h, "wk": wk_h, "wv": wv_h, "wo": wo_h,
                "cosT": cosT, "sinT": sinT,
            })
    return in_maps


def kernel(x, cos, sin, wq, wk, wv, wo, batch=B, window=WINDOW, **_):
    x = np.asarray(x)
    nc = _get_nc()
    in_maps = _prep_inputs(np.asarray(x, np.float32), np.asarray(cos, np.float32),
                           np.asarray(sin, np.float32), np.asarray(wq, np.float32),
                           np.asarray(wk, np.float32), np.asarray(wv, np.float32),
                           np.asarray(wo, np.float32))
    res = run_bass_kernel_spmd(nc, in_maps, core_ids=list(range(N_CORES)))
    out = np.zeros((B * S, DIM), np.float32)
    for g in range(G):
        for s in range(B):
            out[s * S:(s + 1) * S, :] += res.results[g * B + s]["outT"].T
    return out
